# revision 25
# baseline (speedup 1.0000x reference)
"""Trainium2 Bass kernel for nn_Bilinear_54065048322517.

Math:  out[b, j] = input2[b, j] * sum_{i,k} weights[i, j, k] * input1[b, i]
           =   input2 * (input1 @ weights.sum(axis=2))
Shapes: input1 (16384, 64) f32, input2 (16384, 2048) f32,
        weights (64, 2048, 64) f32, out (16384, 2048) f32.

Sharding: split J=2048 into 8 shards of 256 (one per NeuronCore).
Each core reads: input1 full (4MB) + its input2 shard (16MB) + its
weights shard (4MB), writes its out shard (16MB) -> 40MB of HBM
traffic per core (vs 64.5MB for batch sharding, which would have to
replicate the 32MB weights).

Built on bacc.Bacc + TileContext; Bacc.compile() legalizes the
one-embedded-wait-per-instruction TRN2 constraint by splitting extra
waits into event-semaphore instructions.

Per-core kernel (all fp32):
  phase A:
    - weights shard (64,256,64) loaded as 8 chunks (64, 2048) on the
      ACT ring; DVE grouped-reduce over K -> w2dup[0:64] (64, 256);
      one SBUF->SBUF DMA duplicates it to partitions 64-127 so both
      matmul row-groups have an aligned copy.
    - input1 loaded as (128, 8192) on the SP ring: partition p holds
      rows {256n + 2p + q} (512B contiguous runs); 64 TensorE
      (128,128) transposes -> x1T (128, 8192):
      x1T[q*64 + i, n*128 + p] = x1[256n + 2p + q, i].
  phase B (groups of 8 super-tiles = 2048 rows):
    - DMA x2 group -> xtile (128, 4096), SP ring (2KB runs)
    - per super-tile n: 2 matmuls (K=64, M=128, N=256) on DISJOINT
      PE row-groups (q=0 rows 0-63, q=1 rows 64-127, concurrent):
        pt[:, q*256:(q+1)*256] =
            x1T[64q:64q+64, n*128:+128].T @ w2dup[64q:64q+64]
      (psum partition p of q-half <-> row 256n + 2p + q)
    - DVE: otile = pt * xtile
    - DMA otile -> out group, SP ring.
"""

import numpy as np

B, I, J, K = 16384, 64, 2048, 64
NCORES = 8
JS = J // NCORES          # 256 columns per core
NSUP = B // 256           # 64 super-tiles of 256 rows
GROUP = 8                 # super-tiles per DMA group (2MB per stream DMA)
NG = NSUP // GROUP        # 8 groups
NWCHUNK = 8               # weights load chunks
WBUFS = 4                 # weight chunk slots
XBUFS = 3                 # xtile buffer depth
OBUFS = 2                 # otile buffer depth

_CACHE = {}


def _build_nc():
    from contextlib import ExitStack

    import concourse.mybir as mybir
    import concourse.tile as tile
    from concourse import bacc, masks

    f32 = mybir.dt.float32
    nc = bacc.Bacc()

    x1 = nc.dram_tensor("input1", [B, I], f32, kind="ExternalInput")
    x2 = nc.dram_tensor("input2", [B, JS], f32, kind="ExternalInput")
    w = nc.dram_tensor("weights", [I, JS, K], f32, kind="ExternalInput")
    out = nc.dram_tensor("out", [B, JS], f32, kind="ExternalOutput")

    with tile.TileContext(nc) as tc, ExitStack() as ctx:
        const_pool = ctx.enter_context(tc.tile_pool(name="const", bufs=1))
        stage_pool = ctx.enter_context(tc.tile_pool(name="stage", bufs=1))
        wc_pool = ctx.enter_context(tc.tile_pool(name="wc", bufs=WBUFS))
        x_pool = ctx.enter_context(tc.tile_pool(name="xin", bufs=XBUFS))
        o_pool = ctx.enter_context(tc.tile_pool(name="oout", bufs=OBUFS))
        ps_pool = ctx.enter_context(tc.tile_pool(name="ps", bufs=3, space="PSUM"))
        tr_pool = ctx.enter_context(tc.tile_pool(name="tr", bufs=2, space="PSUM"))

        identity = const_pool.tile([128, 128], f32)
        masks.make_identity(nc, identity[:])

        # ---- input1 load (SP ring) ----
        x1stage = stage_pool.tile([128, B * I // 128], f32)  # (128, 8192)
        x1_r = x1.rearrange("(n p q) i -> p n q i", p=128, q=2)  # (128,64,2,64)
        half = B * I // 128 // 2
        nhalf = NSUP // 2
        nc.sync.dma_start(
            out=x1stage[:, 0:half].rearrange("p (n q i) -> p n q i", q=2, i=I),
            in_=x1_r[:, 0:nhalf],
        )
        nc.sync.dma_start(
            out=x1stage[:, half:].rearrange("p (n q i) -> p n q i", q=2, i=I),
            in_=x1_r[:, nhalf:],
        )

        # ---- weights load (ACT ring) + K-reduction -> w2dup ----
        w_flat = w.rearrange("i j k -> i (j k)")  # (64, 16384)
        w2dup = const_pool.tile([128, JS], f32)
        csz = JS * K // NWCHUNK  # elems per chunk per partition
        jcs = JS // NWCHUNK      # w2 columns per chunk
        for c in range(NWCHUNK):
            wchunk = wc_pool.tile([64, csz], f32, name=f"wchunk{c}", tag="wchunk")
            nc.scalar.dma_start(
                out=wchunk[:], in_=w_flat[:, c * csz : (c + 1) * csz]
            )
            nc.vector.tensor_reduce(
                out=w2dup[0:64, c * jcs : (c + 1) * jcs],
                in_=wchunk[:].rearrange("p (j k) -> p j k", k=K),
                axis=mybir.AxisListType.X,
                op=mybir.AluOpType.add,
            )
        # duplicate w2 to partitions 64-127 (aligned rhs for q=1 matmuls)
        nc.scalar.dma_start(out=w2dup[64:128, :], in_=w2dup[0:64, :])

        # ---- transposes: x1T[q*64+i, n*128+p] = x1[256n+2p+q, i] ----
        x1T = const_pool.tile([128, NSUP * 128], f32)  # (128, 8192)
        NB = NSUP // 4  # 16 transpose batches
        for m in range(NB):
            tt = tr_pool.tile([128, 512], f32)
            for s in range(4):
                n = m * 4 + s
                nc.tensor.transpose(
                    tt[:, s * 128 : (s + 1) * 128],
                    x1stage[:, n * 128 : (n + 1) * 128],
                    identity[:],
                )
            nc.scalar.copy(x1T[:, m * 512 : (m + 1) * 512], tt[:])

        # ---- main loop ----
        x2_r = x2.rearrange(
            "(g s p q) j -> g p s q j", g=NG, s=GROUP, p=128, q=2
        )
        out_r = out.rearrange(
            "(g s p q) j -> g p s q j", g=NG, s=GROUP, p=128, q=2
        )

        xtiles = []

        def load(g):
            assert len(xtiles) == g
            xt = x_pool.tile([128, GROUP * 2 * JS], f32, name=f"xt{g}", tag="xt")
            xtiles.append(xt)
            nc.sync.dma_start(
                out=xt[:].rearrange("p (s q j) -> p s q j", s=GROUP, q=2),
                in_=x2_r[g],
            )

        for g in range(min(XBUFS, NG)):
            load(g)

        for g in range(NG):
            xtile = xtiles[g]
            ot = o_pool.tile([128, GROUP * 2 * JS], f32, name=f"ot{g}", tag="ot")
            for s in range(GROUP):
                n = g * GROUP + s
                # 2 banks: each concurrent row-group matmul drains into its
                # own PSUM bank (q=0 -> cols 0:256, q=1 -> cols 512:768)
                pt = ps_pool.tile([128, 4 * JS], f32)  # (128, 1024)
                for q in range(2):
                    nc.tensor.matmul(
                        pt[:, q * 2 * JS : q * 2 * JS + JS],
                        lhsT=x1T[q * 64 : (q + 1) * 64, n * 128 : (n + 1) * 128],
                        rhs=w2dup[q * 64 : (q + 1) * 64, :],
                        start=True,
                        stop=True,
                    )
                nc.vector.tensor_mul(
                    ot[:, s * 512 : (s + 1) * 512].rearrange(
                        "p (q j) -> p q j", q=2
                    ),
                    pt[:].rearrange("p (q j) -> p q j", q=2)[:, :, 0:JS],
                    xtile[:, s * 512 : (s + 1) * 512].rearrange(
                        "p (q j) -> p q j", q=2
                    ),
                )
            nc.sync.dma_start(
                out=out_r[g],
                in_=ot[:].rearrange("p (s q j) -> p s q j", s=GROUP, q=2),
            )
            if g + XBUFS < NG:
                load(g + XBUFS)

    nc.compile()
    return nc


def _get_nc():
    if "nc" not in _CACHE:
        _CACHE["nc"] = _build_nc()
    return _CACHE["nc"]


def _make_in_maps(input1, input2, weights):
    input1 = np.ascontiguousarray(input1, dtype=np.float32)
    in_maps = []
    for c in range(NCORES):
        sl = slice(c * JS, (c + 1) * JS)
        in_maps.append(
            {
                "input1": input1,
                "input2": np.ascontiguousarray(input2[:, sl], dtype=np.float32),
                "weights": np.ascontiguousarray(weights[:, sl, :], dtype=np.float32),
            }
        )
    return in_maps


def run(input1, input2, weights, trace=False, **spmd_kwargs):
    from concourse.bass_utils import run_bass_kernel_spmd

    nc = _get_nc()
    in_maps = _make_in_maps(input1, input2, weights)
    res = run_bass_kernel_spmd(
        nc, in_maps, core_ids=list(range(NCORES)), trace=trace, **spmd_kwargs
    )
    outs = [res.results[c]["out"] for c in range(NCORES)]
    full = np.concatenate(outs, axis=1)
    return full, res


def kernel(input1, input2, weights):
    full, _ = run(input1, input2, weights, trace=False)
    return full


# revision 27
# speedup vs baseline: 1.0072x; 1.0072x over previous
"""Trainium2 Bass kernel for nn_Bilinear_54065048322517.

Math:  out[b, j] = input2[b, j] * sum_{i,k} weights[i, j, k] * input1[b, i]
           =   input2 * (input1 @ weights.sum(axis=2))
Shapes: input1 (16384, 64) f32, input2 (16384, 2048) f32,
        weights (64, 2048, 64) f32, out (16384, 2048) f32.

Sharding: split J=2048 into 8 shards of 256 (one per NeuronCore).
Each core reads: input1 full (4MB) + its input2 shard (16MB) + its
weights shard (4MB), writes its out shard (16MB) -> 40MB of HBM
traffic per core (vs 64.5MB for batch sharding, which would have to
replicate the 32MB weights).

Built on bacc.Bacc + TileContext; Bacc.compile() legalizes the
one-embedded-wait-per-instruction TRN2 constraint by splitting extra
waits into event-semaphore instructions.

Per-core kernel (all fp32):
  phase A (interleaved with phase B groups):
    - weights shard loaded as 8 chunks (128, 1024) on the ACT ring
      with partition 2i+h (h = j-half; (i,h) strides merge to a
      uniform 8192-elem partition stride); DVE grouped-reduce over K
      -> w2tmp (128, 128); 4 small partition-strided DMAs
      de-interleave/duplicate into w2dup (128, 256) where partitions
      q*64+i hold w2[i, :] for both q.
    - input1 loaded as (128, 8192) on the SP ring: partition p holds
      rows {256n + 2p + q} (512B contiguous runs); 64 TensorE
      (128,128) transposes -> x1T (128, 8192):
      x1T[q*64 + i, n*128 + p] = x1[256n + 2p + q, i].
  phase B (groups of 8 super-tiles = 2048 rows), group g processed
  right after transpose batches 2g+2, 2g+3 so PE streams seamlessly:
    - DMA x2 group -> xtile (128, 4096), SP ring (2KB runs)
    - per super-tile n: 2 matmuls (K=64, M=128, N=256) on DISJOINT
      PE row-groups (q=0 rows 0-63, q=1 rows 64-127, concurrent)
      into separate PSUM banks:
        pt[:, q*512:q*512+256] =
            x1T[64q:64q+64, n*128:+128].T @ w2dup[64q:64q+64]
    - DVE: otile = pt * xtile (strided view over the two banks)
    - DMA otile -> out group, SP ring.
"""

import numpy as np

B, I, J, K = 16384, 64, 2048, 64
NCORES = 8
JS = J // NCORES          # 256 columns per core
NSUP = B // 256           # 64 super-tiles of 256 rows
GROUP = 8                 # super-tiles per DMA group (2MB per stream DMA)
NG = NSUP // GROUP        # 8 groups
NWCHUNK = 8               # weights load chunks
WBUFS = 4                 # weight chunk slots
XBUFS = 3                 # xtile buffer depth
OBUFS = 2                 # otile buffer depth

_CACHE = {}


def _build_nc():
    from contextlib import ExitStack

    import concourse.mybir as mybir
    import concourse.tile as tile
    from concourse import bacc, masks

    f32 = mybir.dt.float32
    nc = bacc.Bacc()

    x1 = nc.dram_tensor("input1", [B, I], f32, kind="ExternalInput")
    x2 = nc.dram_tensor("input2", [B, JS], f32, kind="ExternalInput")
    w = nc.dram_tensor("weights", [I, JS, K], f32, kind="ExternalInput")
    out = nc.dram_tensor("out", [B, JS], f32, kind="ExternalOutput")

    with tile.TileContext(nc) as tc, ExitStack() as ctx:
        const_pool = ctx.enter_context(tc.tile_pool(name="const", bufs=1))
        stage_pool = ctx.enter_context(tc.tile_pool(name="stage", bufs=1))
        wc_pool = ctx.enter_context(tc.tile_pool(name="wc", bufs=WBUFS))
        x_pool = ctx.enter_context(tc.tile_pool(name="xin", bufs=XBUFS))
        o_pool = ctx.enter_context(tc.tile_pool(name="oout", bufs=OBUFS))
        ps_pool = ctx.enter_context(tc.tile_pool(name="ps", bufs=3, space="PSUM"))
        tr_pool = ctx.enter_context(tc.tile_pool(name="tr", bufs=2, space="PSUM"))

        identity = const_pool.tile([128, 128], f32)
        masks.make_identity(nc, identity[:])

        # ---- input1 load (SP ring) ----
        x1stage = stage_pool.tile([128, B * I // 128], f32)  # (128, 8192)
        x1_r = x1.rearrange("(n p q) i -> p n q i", p=128, q=2)  # (128,64,2,64)
        half = B * I // 128 // 2
        nhalf = NSUP // 2
        nc.sync.dma_start(
            out=x1stage[:, 0:half].rearrange("p (n q i) -> p n q i", q=2, i=I),
            in_=x1_r[:, 0:nhalf],
        )
        nc.sync.dma_start(
            out=x1stage[:, half:].rearrange("p (n q i) -> p n q i", q=2, i=I),
            in_=x1_r[:, nhalf:],
        )

        # ---- weights load (ACT ring, 128 partitions) + K-reduction ----
        # chunk c: partition 2i+h <- W[i, 128h + c*16 + j'', :], 4KB runs
        w_v = w.rearrange("i (h c j) k -> c i h (j k)", h=2, c=NWCHUNK)
        w2tmp = const_pool.tile([128, JS // 2], f32)  # (128, 128), part 2i+h
        csz = JS * K // NWCHUNK // 2  # 1024 elems per partition per chunk
        jcs = JS // NWCHUNK // 2      # 16 w2tmp columns per chunk
        for c in range(NWCHUNK):
            wchunk = wc_pool.tile([128, csz], f32, name=f"wchunk{c}", tag="wchunk")
            nc.scalar.dma_start(
                out=wchunk[:],
                in_=w_v[c].rearrange("i h f -> (i h) f"),
            )
            nc.vector.tensor_reduce(
                out=w2tmp[:, c * jcs : (c + 1) * jcs],
                in_=wchunk[:].rearrange("p (j k) -> p j k", k=K),
                axis=mybir.AxisListType.X,
                op=mybir.AluOpType.add,
            )
        # de-interleave + duplicate via DRAM bounce (DRAM APs are free-form):
        # w2dup[q*64+i, 128h+j''] = w2tmp[2i+h, j'']
        w2scratch = nc.dram_tensor("w2scratch", [JS // 2, JS // 2], f32)
        nc.scalar.dma_start(out=w2scratch[:, :], in_=w2tmp[:])
        w2dup = const_pool.tile([128, JS], f32)
        w2s_v = w2scratch.rearrange("(i h) j -> i h j", h=2)  # (64, 2, 128)
        for q in range(2):
            nc.scalar.dma_start(
                out=w2dup[q * 64 : (q + 1) * 64, :].rearrange(
                    "p (h j) -> p h j", h=2
                ),
                in_=w2s_v,
            )

        # ---- main: transposes interleaved with group processing ----
        x1T = const_pool.tile([128, NSUP * 128], f32)  # (128, 8192)

        x2_r = x2.rearrange(
            "(g s p q) j -> g p s q j", g=NG, s=GROUP, p=128, q=2
        )
        out_r = out.rearrange(
            "(g s p q) j -> g p s q j", g=NG, s=GROUP, p=128, q=2
        )

        xtiles = []

        def load(g):
            assert len(xtiles) == g
            xt = x_pool.tile([128, GROUP * 2 * JS], f32, name=f"xt{g}", tag="xt")
            xtiles.append(xt)
            nc.sync.dma_start(
                out=xt[:].rearrange("p (s q j) -> p s q j", s=GROUP, q=2),
                in_=x2_r[g],
            )

        def transpose_batch(m):
            tt = tr_pool.tile([128, 512], f32)
            for s in range(4):
                n = m * 4 + s
                nc.tensor.transpose(
                    tt[:, s * 128 : (s + 1) * 128],
                    x1stage[:, n * 128 : (n + 1) * 128],
                    identity[:],
                )
            nc.scalar.copy(x1T[:, m * 512 : (m + 1) * 512], tt[:])

        def process(g):
            xtile = xtiles[g]
            ot = o_pool.tile([128, GROUP * 2 * JS], f32, name=f"ot{g}", tag="ot")
            for s in range(GROUP):
                n = g * GROUP + s
                # 2 banks: each concurrent row-group matmul drains into its
                # own PSUM bank (q=0 -> cols 0:256, q=1 -> cols 512:768)
                pt = ps_pool.tile([128, 4 * JS], f32)  # (128, 1024)
                for q in range(2):
                    nc.tensor.matmul(
                        pt[:, q * 2 * JS : q * 2 * JS + JS],
                        lhsT=x1T[q * 64 : (q + 1) * 64, n * 128 : (n + 1) * 128],
                        rhs=w2dup[q * 64 : (q + 1) * 64, :],
                        start=True,
                        stop=True,
                    )
                nc.vector.tensor_mul(
                    ot[:, s * 512 : (s + 1) * 512].rearrange(
                        "p (q j) -> p q j", q=2
                    ),
                    pt[:].rearrange("p (q j) -> p q j", q=2)[:, :, 0:JS],
                    xtile[:, s * 512 : (s + 1) * 512].rearrange(
                        "p (q j) -> p q j", q=2
                    ),
                )
            nc.sync.dma_start(
                out=out_r[g],
                in_=ot[:].rearrange("p (s q j) -> p s q j", s=GROUP, q=2),
            )
            if g + XBUFS < NG:
                load(g + XBUFS)

        for g in range(min(XBUFS, NG)):
            load(g)

        NB = NSUP // 4  # 16 transpose batches (2 per group)
        for g in range(NG):
            transpose_batch(2 * g)
            transpose_batch(2 * g + 1)
            if g >= 1:
                process(g - 1)
        process(NG - 1)

    nc.compile()
    return nc


def _get_nc():
    if "nc" not in _CACHE:
        _CACHE["nc"] = _build_nc()
    return _CACHE["nc"]


def _make_in_maps(input1, input2, weights):
    input1 = np.ascontiguousarray(input1, dtype=np.float32)
    in_maps = []
    for c in range(NCORES):
        sl = slice(c * JS, (c + 1) * JS)
        in_maps.append(
            {
                "input1": input1,
                "input2": np.ascontiguousarray(input2[:, sl], dtype=np.float32),
                "weights": np.ascontiguousarray(weights[:, sl, :], dtype=np.float32),
            }
        )
    return in_maps


def run(input1, input2, weights, trace=False, **spmd_kwargs):
    from concourse.bass_utils import run_bass_kernel_spmd

    nc = _get_nc()
    in_maps = _make_in_maps(input1, input2, weights)
    res = run_bass_kernel_spmd(
        nc, in_maps, core_ids=list(range(NCORES)), trace=trace, **spmd_kwargs
    )
    outs = [res.results[c]["out"] for c in range(NCORES)]
    full = np.concatenate(outs, axis=1)
    return full, res


def kernel(input1, input2, weights):
    full, _ = run(input1, input2, weights, trace=False)
    return full


# revision 29
# speedup vs baseline: 1.1521x; 1.1439x over previous
"""Trainium2 Bass kernel for nn_Bilinear_54065048322517.

Math:  out[b, j] = input2[b, j] * sum_{i,k} weights[i, j, k] * input1[b, i]
           =   input2 * (input1 @ weights.sum(axis=2))
Shapes: input1 (16384, 64) f32, input2 (16384, 2048) f32,
        weights (64, 2048, 64) f32, out (16384, 2048) f32.

Sharding: split J=2048 into 8 shards of 256 (one per NeuronCore).
Each core reads: input1 full (4MB) + its input2 shard (16MB) + its
weights shard (4MB), writes its out shard (16MB) -> 40MB of HBM
traffic per core (vs 64.5MB for batch sharding, which would have to
replicate the 32MB weights).

Built on bacc.Bacc + TileContext; Bacc.compile() legalizes the
one-embedded-wait-per-instruction TRN2 constraint by splitting extra
waits into event-semaphore instructions.

Per-core kernel (all fp32):
  phase A (interleaved with phase B groups):
    - weights shard loaded as 8 chunks (128, 1024) on the ACT ring
      with partition 2i+h (h = j-half; (i,h) strides merge to a
      uniform 8192-elem partition stride); two DVE grouped-reduces
      over K -> w2tmp (128, 128); two permutation-matrix matmuls
      P_h.T @ w2tmp (P_h[2i+h, i] = P_h[2i+h, 64+i] = 1) + ACT
      copies de-interleave/duplicate into w2dup (128, 256) where
      partition q*64+i holds w2[i, :] for both q.
    - input1 loaded as 8 chunks (128, 1024) on the SP ring,
      interleaved with the first x2 group loads: partition p holds
      rows {256n + 2p + q} (512B contiguous runs); 64 TensorE
      (128,128) transposes -> x1T (128, 8192):
      x1T[q*64 + i, n*128 + p] = x1[256n + 2p + q, i].
  phase B (groups of 8 super-tiles = 2048 rows), group g processed
  right after transpose batches 2g+2, 2g+3 so PE streams seamlessly:
    - DMA x2 group -> xtile (128, 4096), SP ring (2KB runs)
    - per super-tile n: 2 matmuls (K=64, M=128, N=256) on DISJOINT
      PE row-groups (q=0 rows 0-63, q=1 rows 64-127, concurrent)
      into separate PSUM banks:
        pt[:, q*512:q*512+256] =
            x1T[64q:64q+64, n*128:+128].T @ w2dup[64q:64q+64]
    - DVE: otile = pt * xtile (strided view over the two banks)
    - DMA otile -> out group in two 1MB halves, SP ring.
"""

import numpy as np

B, I, J, K = 16384, 64, 2048, 64
NCORES = 8
JS = J // NCORES          # 256 columns per core
NSUP = B // 256           # 64 super-tiles of 256 rows
GROUP = 8                 # super-tiles per DMA group
NG = NSUP // GROUP        # 8 groups
NWCHUNK = 8               # weights load chunks (all resident)
NXCHUNK = 8               # input1 load chunks
XBUFS = 3                 # xtile buffer depth
OBUFS = 2                 # otile buffer depth

_CACHE = {}


def _build_nc():
    from contextlib import ExitStack

    import concourse.mybir as mybir
    import concourse.tile as tile
    from concourse import bacc, masks

    f32 = mybir.dt.float32
    nc = bacc.Bacc()

    x1 = nc.dram_tensor("input1", [B, I], f32, kind="ExternalInput")
    x2 = nc.dram_tensor("input2", [B, JS], f32, kind="ExternalInput")
    w = nc.dram_tensor("weights", [I, JS, K], f32, kind="ExternalInput")
    out = nc.dram_tensor("out", [B, JS], f32, kind="ExternalOutput")

    with tile.TileContext(nc) as tc, ExitStack() as ctx:
        const_pool = ctx.enter_context(tc.tile_pool(name="const", bufs=1))
        stage_pool = ctx.enter_context(tc.tile_pool(name="stage", bufs=1))
        wc_pool = ctx.enter_context(tc.tile_pool(name="wc", bufs=1))
        x_pool = ctx.enter_context(tc.tile_pool(name="xin", bufs=XBUFS))
        o_pool = ctx.enter_context(tc.tile_pool(name="oout", bufs=OBUFS))
        ps_pool = ctx.enter_context(tc.tile_pool(name="ps", bufs=3, space="PSUM"))
        tr_pool = ctx.enter_context(tc.tile_pool(name="tr", bufs=2, space="PSUM"))

        identity = const_pool.tile([128, 128], f32)
        masks.make_identity(nc, identity[:])

        # permutation masks: P[h][2i+h, i] = P[h][2i+h, 64+i] = 1, else 0
        # (P_h.T @ w2tmp)[q*64+i, j''] = w2tmp[2i+h, j'']
        perm = []
        for h in range(2):
            ph = const_pool.tile([128, 128], f32, name=f"perm{h}")
            perm.append(ph)
            nc.gpsimd.memset(ph[:], 0.0)
            for q in range(2):
                # select p - 2*m - h == 0 over the (128, 64) column block
                nc.gpsimd.affine_select(
                    out=ph[:, q * 64 : (q + 1) * 64],
                    in_=ph[:, q * 64 : (q + 1) * 64],
                    compare_op=mybir.AluOpType.not_equal,
                    fill=1.0,
                    base=-h,
                    pattern=[[-2, 64]],
                    channel_multiplier=1,
                )

        # ---- input1 chunk loads (SP ring) + x2 prefetch interleave ----
        x1stage = stage_pool.tile([128, B * I // 128], f32)  # (128, 8192)
        x1_r = x1.rearrange("(n p q) i -> p n q i", p=128, q=2)  # (128,64,2,64)
        xcsz = B * I // 128 // NXCHUNK  # 1024 elems/partition per chunk
        xnsz = NSUP // NXCHUNK          # 8 super-tiles per chunk

        def load_x1_chunk(k):
            nc.sync.dma_start(
                out=x1stage[:, k * xcsz : (k + 1) * xcsz].rearrange(
                    "p (n q i) -> p n q i", q=2, i=I
                ),
                in_=x1_r[:, k * xnsz : (k + 1) * xnsz],
            )

        x2_r = x2.rearrange(
            "(g s p q) j -> g p s q j", g=NG, s=GROUP, p=128, q=2
        )
        out_r = out.rearrange(
            "(g sh s p q) j -> g sh p s q j", g=NG, sh=2, s=GROUP // 2, p=128, q=2
        )

        xtiles = []

        def load(g):
            assert len(xtiles) == g
            xt = x_pool.tile([128, GROUP * 2 * JS], f32, name=f"xt{g}", tag="xt")
            xtiles.append(xt)
            nc.sync.dma_start(
                out=xt[:].rearrange("p (s q j) -> p s q j", s=GROUP, q=2),
                in_=x2_r[g],
            )

        for k in range(4):
            load_x1_chunk(k)
        load(0)
        for k in range(4, NXCHUNK):
            load_x1_chunk(k)
        load(1)
        load(2)

        # ---- weights load (ACT ring, 128 partitions) + K-reduction ----
        # chunk c: partition 2i+h <- W[i, 128h + c*16 + j'', :], 4KB runs
        w_v = w.rearrange("i (h c j) k -> c i h (j k)", h=2, c=NWCHUNK)
        w2tmp = const_pool.tile([128, JS // 2], f32)  # (128, 128), part 2i+h
        csz = JS * K // NWCHUNK // 2  # 1024 elems per partition per chunk
        jcs = JS // NWCHUNK // 2      # 16 w2tmp columns per chunk
        wchunks = []
        for c in range(NWCHUNK):
            wchunk = wc_pool.tile(
                [128, csz], f32, name=f"wchunk{c}", tag=f"wchunk{c}"
            )
            wchunks.append(wchunk)
            nc.scalar.dma_start(
                out=wchunk[:],
                in_=w_v[c].rearrange("i h f -> (i h) f"),
            )
        # two big reduces (fewer DVE drains); chunks are column-adjacent
        # in w2tmp and contiguous SBUF slots are NOT guaranteed, so reduce
        # each chunk's 16-column slice but batch 4 per instruction via AP?
        # -> keep it simple: one reduce per chunk-pair is not possible
        #    across tiles; do per-chunk reduces into w2tmp.
        for c in range(NWCHUNK):
            nc.vector.tensor_reduce(
                out=w2tmp[:, c * jcs : (c + 1) * jcs],
                in_=wchunks[c][:].rearrange("p (j k) -> p j k", k=K),
                axis=mybir.AxisListType.X,
                op=mybir.AluOpType.add,
            )

        # de-interleave + duplicate via permutation matmuls:
        # w2dup[q*64+i, 128h+j''] = w2tmp[2i+h, j'']
        w2dup = const_pool.tile([128, JS], f32)
        for h in range(2):
            pw = tr_pool.tile([128, 512], f32, name="tt", tag="tt")
            nc.tensor.matmul(
                pw[:, 0:128], lhsT=perm[h][:], rhs=w2tmp[:],
                start=True, stop=True,
            )
            nc.scalar.copy(w2dup[:, h * 128 : (h + 1) * 128], pw[:, 0:128])

        # ---- transposes + groups, interleaved ----
        x1T = const_pool.tile([128, NSUP * 128], f32)  # (128, 8192)

        def transpose_batch(m):
            tt = tr_pool.tile([128, 512], f32, name="tt", tag="tt")
            for s in range(4):
                n = m * 4 + s
                nc.tensor.transpose(
                    tt[:, s * 128 : (s + 1) * 128],
                    x1stage[:, n * 128 : (n + 1) * 128],
                    identity[:],
                )
            nc.scalar.copy(x1T[:, m * 512 : (m + 1) * 512], tt[:])

        def process(g):
            xtile = xtiles[g]
            ot = o_pool.tile([128, GROUP * 2 * JS], f32, name=f"ot{g}", tag="ot")
            for s in range(GROUP):
                n = g * GROUP + s
                # 2 banks: each concurrent row-group matmul drains into its
                # own PSUM bank (q=0 -> cols 0:256, q=1 -> cols 512:768)
                pt = ps_pool.tile([128, 4 * JS], f32)  # (128, 1024)
                for q in range(2):
                    nc.tensor.matmul(
                        pt[:, q * 2 * JS : q * 2 * JS + JS],
                        lhsT=x1T[q * 64 : (q + 1) * 64, n * 128 : (n + 1) * 128],
                        rhs=w2dup[q * 64 : (q + 1) * 64, :],
                        start=True,
                        stop=True,
                    )
                nc.vector.tensor_mul(
                    ot[:, s * 512 : (s + 1) * 512].rearrange(
                        "p (q j) -> p q j", q=2
                    ),
                    pt[:].rearrange("p (q j) -> p q j", q=2)[:, :, 0:JS],
                    xtile[:, s * 512 : (s + 1) * 512].rearrange(
                        "p (q j) -> p q j", q=2
                    ),
                )
                if s == GROUP // 2 - 1 or s == GROUP - 1:
                    sh = 0 if s < GROUP // 2 else 1
                    nc.sync.dma_start(
                        out=out_r[g, sh],
                        in_=ot[
                            :, sh * GROUP * JS : (sh + 1) * GROUP * JS
                        ].rearrange("p (s q j) -> p s q j", s=GROUP // 2, q=2),
                    )
            if g + XBUFS < NG:
                load(g + XBUFS)

        for g in range(NG):
            transpose_batch(2 * g)
            transpose_batch(2 * g + 1)
            if g >= 1:
                process(g - 1)
        process(NG - 1)

    nc.compile()
    return nc


def _get_nc():
    if "nc" not in _CACHE:
        _CACHE["nc"] = _build_nc()
    return _CACHE["nc"]


def _make_in_maps(input1, input2, weights):
    input1 = np.ascontiguousarray(input1, dtype=np.float32)
    in_maps = []
    for c in range(NCORES):
        sl = slice(c * JS, (c + 1) * JS)
        in_maps.append(
            {
                "input1": input1,
                "input2": np.ascontiguousarray(input2[:, sl], dtype=np.float32),
                "weights": np.ascontiguousarray(weights[:, sl, :], dtype=np.float32),
            }
        )
    return in_maps


def run(input1, input2, weights, trace=False, **spmd_kwargs):
    from concourse.bass_utils import run_bass_kernel_spmd

    nc = _get_nc()
    in_maps = _make_in_maps(input1, input2, weights)
    res = run_bass_kernel_spmd(
        nc, in_maps, core_ids=list(range(NCORES)), trace=trace, **spmd_kwargs
    )
    outs = [res.results[c]["out"] for c in range(NCORES)]
    full = np.concatenate(outs, axis=1)
    return full, res


def kernel(input1, input2, weights):
    full, _ = run(input1, input2, weights, trace=False)
    return full
